# revision 42
# baseline (speedup 1.0000x reference)
"""TRN2 Bass kernel for nn_DiffQuantumSimulator (QAOA MaxCut, 18 qubits, p=4).

Strategy: data-parallel over batch (8 graphs -> 8 NeuronCores). Per core the
2^18 statevector lives in SBUF as [128 partitions x 2048] (re/im fp16 split).

Each QAOA layer applies exp(-i*hp) (diagonal, elementwise) and the mixer
RX(beta)^(x)18 done in 3 TensorE matmul phases:
  A: 128x128 complex gate RX^(x)7 on the 7 partition bits, fused with a
     partition<->free-bit transpose by using the *state* as the stationary
     operand (out = state_tile^T @ [C|D]).
  B: same trick on the next 7 bits (stride-16 windows).
  C: standard matmul applying RX^(x)4 (x) I_8 to the remaining 4 bits.

All matmuls run in fp16 (triggers fast weight load, full PE rate). PSUM
accumulates fp32. Phase epilogues copy PSUM->SBUF fp16 on Act/DVE; the
rotation (complex multiply by exp(-i*hp)) then runs on SBUF fp16 operands at
DVE 2x rate, with one multiply per chunk offloaded to GpSimd. B and C phases
are interleaved on the PE so the next layer's rotation starts early.

Diagonals (cos/sin of hp per layer layout), gate matrices and the MaxCut
diagonal hp are precomputed on host from the runtime inputs. Device returns
per-partition energy partial sums; host reduces and scales.
"""

import numpy as np

import concourse.bass as bass
import concourse.mybir as mybir
import concourse.tile as tile
from concourse import bacc
from concourse.bass_utils import run_bass_kernel_spmd

N = 18
DIM = 1 << N
P = 128
F = DIM // P  # 2048
LAYERS = 4
BATCH = 8
NCORES = 8

FP32 = mybir.dt.float32
FP16 = mybir.dt.float16
ALU = mybir.AluOpType
ACT = mybir.ActivationFunctionType

# ----------------------------------------------------------------------------
# Host-side math: hp diagonal, gate matrices, bit-layout permutations
# ----------------------------------------------------------------------------


def _compute_hp(adj):
    W = (np.triu(adj, k=1) > 0.5).astype(np.float64)
    n_edges = W.sum()
    idx = np.arange(DIM)
    shifts = (N - 1 - np.arange(N))[:, None]
    Z = 1.0 - 2.0 * ((idx[None, :] >> shifts) & 1).astype(np.float64)
    T = W @ Z
    cross = np.einsum("ud,ud->d", T, Z)
    return 0.5 * (n_edges - cross)  # [DIM], integer-valued, exact


def _rx(beta):
    c, s = np.cos(beta), np.sin(beta)
    return np.array([[c, -1j * s], [-1j * s, c]], dtype=np.complex128)


def _kron_list(mats):
    out = np.array([[1.0]], dtype=np.complex128)
    for m in mats:
        out = np.kron(out, m)
    return out


def _m7(beta):
    return _kron_list([_rx(beta)] * 7)


def _m41(beta):
    return _kron_list([_rx(beta)] * 4 + [np.eye(2, dtype=np.complex128)] * 3)


def _bitmap_after_A(bm):
    new = [0] * N
    for j in range(7):
        new[11 + j] = bm[j]
    for j in range(4):
        new[7 + j] = bm[7 + j]
    for j in range(7):
        new[j] = bm[11 + j]
    return new


def _bitmap_after_B(bm):
    # window = free bits 10..4 (stride-16 single AP dim), tiles = bits 3..0
    new = [0] * N
    for j in range(7):
        new[11 + j] = bm[4 + j]
    for j in range(4):
        new[7 + j] = bm[j]
    for j in range(7):
        new[j] = bm[11 + j]
    return new


def _perm_for_bitmap(bm):
    a = np.arange(DIM, dtype=np.int64)
    out = np.zeros(DIM, dtype=np.int64)
    for j in range(N):
        out |= ((a >> j) & 1) << bm[j]
    return out


def _layer_perms():
    """Permutations (orig_idx = perm[cur_idx]) for the state layout at the
    start of each layer (1..LAYERS) plus the final layout (index LAYERS)."""
    perms = []
    bm = list(range(N))
    for _ in range(LAYERS):
        perms.append(_perm_for_bitmap(bm))
        bm = _bitmap_after_B(_bitmap_after_A(bm))
    perms.append(_perm_for_bitmap(bm))
    return perms


_PERMS = _layer_perms()


def _host_prep(batch_betas, adj_matrices):
    """Build per-core input dicts."""
    in_maps = []
    for b in range(BATCH):
        hp = _compute_hp(np.asarray(adj_matrices[b], dtype=np.float64))
        cos_hp = np.cos(hp)
        sin_hp = np.sin(hp)

        init_re = cos_hp[_PERMS[0]].astype(np.float16).reshape(P, F)
        init_im = (-sin_hp[_PERMS[0]]).astype(np.float16).reshape(P, F)

        # all diags packed p-major: [P, 7*F], diag k at cols [k*F:(k+1)*F]
        n_diag = 2 * (LAYERS - 1) + 1
        diags = np.empty((P, n_diag * F), dtype=np.float16)
        for t in range(1, LAYERS):
            diags[:, (2 * (t - 1)) * F : (2 * t - 1) * F] = (
                cos_hp[_PERMS[t]].astype(np.float16).reshape(P, F)
            )
            diags[:, (2 * (t - 1) + 1) * F : (2 * t) * F] = (
                sin_hp[_PERMS[t]].astype(np.float16).reshape(P, F)
            )
        diags[:, (n_diag - 1) * F :] = hp[_PERMS[LAYERS]].astype(np.float16).reshape(P, F)

        gates_ab = np.empty((P, LAYERS * 512), dtype=np.float16)
        gates_c = np.empty((P, LAYERS * 384), dtype=np.float16)
        for t in range(LAYERS):
            beta = float(np.asarray(batch_betas[b][t], dtype=np.float64))
            M7 = _m7(beta)
            C7 = M7.real.astype(np.float16)
            D7 = M7.imag.astype(np.float16)
            M41 = _m41(beta)
            C41 = M41.real.astype(np.float16)
            D41 = M41.imag.astype(np.float16)
            o = 512 * t
            gates_ab[:, o : o + 128] = C7
            gates_ab[:, o + 128 : o + 256] = D7
            gates_ab[:, o + 256 : o + 384] = -D7
            gates_ab[:, o + 384 : o + 512] = C7
            o = 384 * t
            gates_c[:, o : o + 128] = C41
            gates_c[:, o + 128 : o + 256] = -D41
            gates_c[:, o + 256 : o + 384] = D41

        in_maps.append(
            {
                "init_re": init_re,
                "init_im": init_im,
                "diags": diags,
                "gates_ab": gates_ab,
                "gates_c": gates_c,
            }
        )
    return in_maps


# ----------------------------------------------------------------------------
# Bass program
# ----------------------------------------------------------------------------


def _build_program():
    nc = bacc.Bacc("TRN2", target_bir_lowering=False, debug=False)

    n_diag = 2 * (LAYERS - 1) + 1
    d_init_re = nc.dram_tensor("init_re", [P, F], FP16, kind="ExternalInput")
    d_init_im = nc.dram_tensor("init_im", [P, F], FP16, kind="ExternalInput")
    d_diags = nc.dram_tensor("diags", [P, n_diag * F], FP16, kind="ExternalInput")
    d_gates_ab = nc.dram_tensor("gates_ab", [P, LAYERS * 512], FP16, kind="ExternalInput")
    d_gates_c = nc.dram_tensor("gates_c", [P, LAYERS * 384], FP16, kind="ExternalInput")
    d_out = nc.dram_tensor("out", [P, 8], FP32, kind="ExternalOutput")

    with tile.TileContext(nc) as tc:
        with (
            tc.tile_pool(name="state", bufs=1) as st_pool,
            tc.tile_pool(name="consts", bufs=1) as c_pool,
            tc.tile_pool(name="scratch", bufs=1) as s_pool,
            tc.tile_pool(name="ps", bufs=4, space="PSUM") as ps_pool,
        ):
            # state buffers (fp16). ab/bc keep the raw PSUM layout
            # (g, j, reim, n) so phase epilogues are verbatim copies.
            re_a = st_pool.tile([P, F], FP16, tag="re_a")
            im_a = st_pool.tile([P, F], FP16, tag="im_a")
            re_b = st_pool.tile([P, F], FP16, tag="re_b")
            im_b = st_pool.tile([P, F], FP16, tag="im_b")
            bc = st_pool.tile([P, 2 * F], FP16, tag="bc")
            # phase-C output, fp16, [pre | pim] per 512-chunk
            cri = [
                st_pool.tile([P, 1024], FP16, tag=f"cri{k}", name=f"cri{k}")
                for k in range(4)
            ]

            diag_all = c_pool.tile([P, n_diag * F], FP16, tag="diags")
            gab_all = c_pool.tile([P, LAYERS * 512], FP16, tag="gab")
            gc_all = c_pool.tile([P, LAYERS * 384], FP16, tag="gc")

            # rotation scratch: 2 parities x (rs0, rs1, rs2, rs3)
            rs = [
                [
                    s_pool.tile([P, 512], FP16, tag=f"rs{p}{k}", name=f"rs{p}{k}")
                    for k in range(4)
                ]
                for p in range(2)
            ]
            # energy scratch
            sq = [
                s_pool.tile([P, 1024], FP32, tag=f"sq{k}", name=f"sq{k}")
                for k in range(4)
            ]
            junk = s_pool.tile([P, 512], FP32, tag="junk")
            part8 = s_pool.tile([P, 8], FP32, tag="part8")

            # ---- input DMAs: few large triggers (each costs ~650ns on the
            # issuing sequencer), layer-0/chunk-0 slices first so compute
            # starts early; descriptors fan out over all 16 DMA engines
            nc.gpsimd.dma_start(re_a[:, 0:512], d_init_re.ap()[:, 0:512])
            nc.sync.dma_start(gab_all[:, 0:512], d_gates_ab.ap()[:, 0:512])
            nc.gpsimd.dma_start(im_a[:, 0:512], d_init_im.ap()[:, 0:512])
            nc.sync.dma_start(gc_all[:, 0:384], d_gates_c.ap()[:, 0:384])
            nc.gpsimd.dma_start(re_a[:, 512:F], d_init_re.ap()[:, 512:F])
            nc.sync.dma_start(im_a[:, 512:F], d_init_im.ap()[:, 512:F])
            nc.gpsimd.dma_start(
                gab_all[:, 512 : LAYERS * 512], d_gates_ab.ap()[:, 512 : LAYERS * 512]
            )
            nc.sync.dma_start(
                gc_all[:, 384 : LAYERS * 384], d_gates_c.ap()[:, 384 : LAYERS * 384]
            )
            nc.gpsimd.dma_start(diag_all[:, 0 : 2 * F], d_diags.ap()[:, 0 : 2 * F])
            nc.sync.dma_start(
                diag_all[:, 2 * F : n_diag * F], d_diags.ap()[:, 2 * F : n_diag * F]
            )

            def rot_chunk(t, k):
                """state[chunk k] = cri[k] * exp(-i hp) -> re_a/im_a (fp16).

                DVE: rs0=pre*cs, rs1=pim*sn, re'=rs0+rs1, rs3=pim*cs,
                     im'=rs3-rs2;  Pool: rs2=pre*sn.
                """
                pre = cri[k][:, 0:512]
                pim = cri[k][:, 512:1024]
                co = (2 * (t - 1)) * F + 512 * k
                so = (2 * (t - 1) + 1) * F + 512 * k
                cs = diag_all[:, co : co + 512]
                sn = diag_all[:, so : so + 512]
                r = rs[k % 2]
                dst_re = re_a[:, 512 * k : 512 * (k + 1)]
                dst_im = im_a[:, 512 * k : 512 * (k + 1)]
                # re-path first: phase A's re-matmuls unblock one op earlier
                nc.vector.tensor_tensor(r[0][:], pre, cs, ALU.mult)
                nc.vector.tensor_tensor(r[1][:], pim, sn, ALU.mult)
                nc.vector.tensor_tensor(dst_re, r[0][:], r[1][:], ALU.add)
                nc.vector.tensor_tensor(r[2][:], pre, sn, ALU.mult)
                nc.vector.tensor_tensor(r[3][:], pim, cs, ALU.mult)
                nc.vector.tensor_tensor(dst_im, r[3][:], r[2][:], ALU.subtract)

            def phase_a_group(t, g):
                cd7 = gab_all[:, 512 * t : 512 * t + 256]
                ndc7 = gab_all[:, 512 * t + 256 : 512 * t + 512]
                ps = ps_pool.tile([P, 1024], FP32, tag="ps")
                for j in range(4):
                    w = 4 * g + j
                    sl = slice(128 * w, 128 * (w + 1))
                    out_sl = ps[:, 256 * j : 256 * (j + 1)]
                    nc.tensor.matmul(out_sl, re_a[:, sl], cd7, start=True, stop=False)
                    nc.tensor.matmul(out_sl, im_a[:, sl], ndc7, start=False, stop=True)
                src = ps[:].rearrange("p (j h) -> p j h", j=4)
                dst = slice(512 * g, 512 * (g + 1))
                nc.scalar.copy(re_b[:, dst], src[:, :, 0:128])
                if g < 3:
                    nc.scalar.copy(im_b[:, dst], src[:, :, 128:256])
                else:
                    # last group gates phase B: split across Act || DVE
                    nc.vector.tensor_copy(im_b[:, dst], src[:, :, 128:256])

            def phase_b_group(t, g):
                cd7 = gab_all[:, 512 * t : 512 * t + 256]
                ndc7 = gab_all[:, 512 * t + 256 : 512 * t + 512]
                re_b4 = re_b[:].rearrange("p (w u) -> p w u", w=128)
                im_b4 = im_b[:].rearrange("p (w u) -> p w u", w=128)
                ps = ps_pool.tile([P, 1024], FP32, tag="ps")
                for j in range(4):
                    w = 4 * g + j
                    out_sl = ps[:, 256 * j : 256 * (j + 1)]
                    nc.tensor.matmul(
                        out_sl, re_b4[:, :, w], cd7, start=True, stop=False
                    )
                    nc.tensor.matmul(
                        out_sl, im_b4[:, :, w], ndc7, start=False, stop=True
                    )
                dst = slice(1024 * g, 1024 * (g + 1))
                if g == 0:
                    # g0 gates C chunk 0 -> next layer's rotation: split
                    src = ps[:].rearrange("p (j h) -> p j h", j=4)
                    bcv = bc[:, dst].rearrange("p (j r n) -> p j r n", j=4, r=2)
                    nc.scalar.copy(bcv[:, :, 0, :], src[:, :, 0:128])
                    nc.vector.tensor_copy(bcv[:, :, 1, :], src[:, :, 128:256])
                elif g == 2:
                    nc.scalar.copy(bc[:, dst], ps[:])
                else:
                    nc.vector.tensor_copy(bc[:, dst], ps[:])

            def phase_c_chunk(t, k):
                c41 = gc_all[:, 384 * t : 384 * t + 128]
                nd41 = gc_all[:, 384 * t + 128 : 384 * t + 256]
                d41 = gc_all[:, 384 * t + 256 : 384 * t + 384]
                bc_v = bc[:].rearrange("p (g j r n) -> p g j r n", g=4, j=4, r=2)
                mv_re = bc_v[:, k, :, 0, :]
                mv_im = bc_v[:, k, :, 1, :]
                ps = ps_pool.tile([P, 1024], FP32, tag="ps")
                pre = ps[:, 0:512]
                pim = ps[:, 512:1024]
                nc.tensor.matmul(pre, c41, mv_re, start=True, stop=False)
                nc.tensor.matmul(pim, c41, mv_im, start=True, stop=False)
                nc.tensor.matmul(pre, nd41, mv_im, start=False, stop=True)
                nc.tensor.matmul(pim, d41, mv_re, start=False, stop=True)
                if t < LAYERS - 1:
                    # fp32 PSUM -> fp16 SBUF, consumed by next layer's rotation
                    if k == 0:
                        nc.scalar.copy(cri[k][:, 0:512], pre)
                        nc.vector.tensor_copy(cri[k][:, 512:1024], pim)
                    else:
                        nc.scalar.copy(cri[k][:], ps[:])
                else:
                    # energy: sum(|amp|^2 * hp) per partition
                    ho = (n_diag - 1) * F + 512 * k
                    hp_ck = diag_all[:, ho : ho + 512]
                    nc.scalar.activation(sq[k][:], ps[:], ACT.Square)
                    nc.vector.scalar_tensor_tensor(
                        junk[:], sq[k][:, 0:512], 1.0, hp_ck,
                        ALU.mult, ALU.mult,
                        accum_out=part8[:, 2 * k : 2 * k + 1],
                    )
                    nc.vector.scalar_tensor_tensor(
                        junk[:], sq[k][:, 512:1024], 1.0, hp_ck,
                        ALU.mult, ALU.mult,
                        accum_out=part8[:, 2 * k + 1 : 2 * k + 2],
                    )

            for t in range(LAYERS):
                # rotation + phase A, chunk-pipelined
                for g in range(4):
                    if t > 0:
                        rot_chunk(t, g)
                    phase_a_group(t, g)
                # phases B and C interleaved on the PE so C chunk 0 (and the
                # next layer's rotation) starts while B is still running
                phase_b_group(t, 0)
                phase_b_group(t, 1)
                phase_c_chunk(t, 0)
                phase_b_group(t, 2)
                phase_b_group(t, 3)
                phase_c_chunk(t, 1)
                phase_c_chunk(t, 2)
                phase_c_chunk(t, 3)

            nc.sync.dma_start(d_out.ap(), part8[:])

    nc.compile()
    return nc


_NC_CACHE = {}


def _get_program():
    if "nc" not in _NC_CACHE:
        _NC_CACHE["nc"] = _build_program()
    return _NC_CACHE["nc"]


def kernel(batch_betas, adj_matrices, _trace=False, _tmpdir=None):
    batch_betas = np.asarray(batch_betas, dtype=np.float32)
    adj_matrices = np.asarray(adj_matrices, dtype=np.float32)
    assert batch_betas.shape == (BATCH, LAYERS)
    assert adj_matrices.shape == (BATCH, N, N)

    nc = _get_program()
    in_maps = _host_prep(batch_betas, adj_matrices)
    res = run_bass_kernel_spmd(
        nc,
        in_maps,
        list(range(NCORES)),
        trace=_trace,
        tmpdir=_tmpdir,
    )
    energies = np.array(
        [res.results[b]["out"].sum() / DIM for b in range(BATCH)], dtype=np.float32
    )
    if _trace:
        return energies, res
    return energies
